# revision 2
# baseline (speedup 1.0000x reference)
"""HP_AGG grid message-passing kernel for 8 Trainium2 NeuronCores.

Reference op: out = (index_mask @ feats) / divide_num  per batch, with
  feats [B=16, N=4096, C=384], index_mask [N, N], divide_num [N, 1].

index_mask is a 3x3-window grid adjacency on a 64x64 grid, so the scaled
operator M = index_mask / divide_num is block-tridiagonal in 128-row node
blocks (bandwidth 65 < 128).  The kernel never ships the 67 MB mask to the
device: it slices M into 128x128 blocks host-side, dedupes them (5 unique
matrices for the grid adjacency), and computes each output block as a sum
of <=3 TensorEngine matmuls accumulated in PSUM:

    out[m] = sum_j  W[m, m+j].T @ feats[m+j]      (W built from the inputs)

Precision/bandwidth: feats and W are cast to fp16 host-side (PE runs fp16
at 4x the fp32 rate and input HBM traffic halves); accumulation stays in
fp32 PSUM; the result is written back as fp16 and upcast to fp32 on the
host.  End-to-end max rel err vs the fp32 reference is ~6e-4.

Sharding: data-parallel over batch, 2 batches per core.  Per-core HBM
traffic is 2*3.15 MB in + 2*3.15 MB out + 0.16 MB weights ~= 12.7 MB
=> ~35 us DMA roofline; PE does 188 fp16 matmuls ~= 30 us, overlapped.
"""

import numpy as np

import concourse.bacc as bacc
import concourse.mybir as mybir
from concourse import bass_utils
from concourse.tile import TileContext

B, N, C = 16, 4096, 384
P = 128                 # partition count == node-block size
NCORES = 8
BPC = B // NCORES       # batches per core
NBLK = N // P           # 32 node blocks
STRIP = 4               # node blocks per PSUM strip (4 banks of 512 fp32)
NSTRIP = NBLK // STRIP
CHUNK = 8               # node blocks per input DMA chunk (786 KB fp16)
NCHUNK = NBLK // CHUNK
F16 = mybir.dt.float16
F32 = mybir.dt.float32

LAST = None             # BassKernelResults of the most recent run (for test.py)


def _build(blocks, n_uniq):
    """Trace the SPMD program.  blocks: {m: [(mj, uid), ...]} sorted by mj."""
    nc = bacc.Bacc("TRN2", target_bir_lowering=False, debug=False)
    feats_t = nc.dram_tensor("feats", [BPC, N, C], F16, kind="ExternalInput")
    wgts_t = nc.dram_tensor("wgts", [n_uniq, P, P], F16, kind="ExternalInput")
    out_t = nc.dram_tensor("out", [BPC, N, C], F16, kind="ExternalOutput")

    with TileContext(nc) as tc:
        with (
            tc.tile_pool(name="wpool", bufs=1) as wpool,
            tc.tile_pool(name="fpool", bufs=2) as fpool,
            tc.tile_pool(name="opool", bufs=3) as opool,
            tc.tile_pool(name="ppool", bufs=2, space="PSUM") as ppool,
        ):
            wtile = wpool.tile([P, n_uniq, P], F16, tag="w")
            nc.sync.dma_start(
                out=wtile[:, :, :], in_=wgts_t.rearrange("u k m -> k u m")
            )

            for b in range(BPC):
                fchunks = []
                for ci in range(NCHUNK):
                    fc = fpool.tile([P, CHUNK, C], F16, name=f"f{b}_{ci}",
                                    tag=f"f{ci}")
                    rows = feats_t[b, ci * CHUNK * P : (ci + 1) * CHUNK * P, :]
                    nc.sync.dma_start(
                        out=fc[:, :, :],
                        in_=rows.rearrange("(s p) c -> p s c", p=P),
                    )
                    fchunks.append(fc)

                for s in range(NSTRIP):
                    ptile = ppool.tile([P, STRIP, 512], F32, name=f"p{b}_{s}", tag="p")
                    for k in range(STRIP):
                        lst = blocks[s * STRIP + k]
                        for idx, (mj, uid) in enumerate(lst):
                            fc = fchunks[mj // CHUNK]
                            nc.tensor.matmul(
                                ptile[:, k, 0:C],
                                wtile[:, uid, :],
                                fc[:, mj % CHUNK, :],
                                start=(idx == 0),
                                stop=(idx == len(lst) - 1),
                            )
                    otile = opool.tile([P, STRIP, C], F16, name=f"o{b}_{s}", tag="o")
                    nc.vector.tensor_copy(out=otile[:, :, :], in_=ptile[:, :, 0:C])
                    dst = out_t[b, s * STRIP * P : (s + 1) * STRIP * P, :]
                    nc.scalar.dma_start(
                        out=dst.rearrange("(s p) c -> p s c", p=P),
                        in_=otile[:, :, :],
                    )
    nc.compile()
    return nc


def _prep_weights(index_mask, divide_num):
    """Slice M = index_mask/divide_num into nonzero 128x128 blocks, deduped."""
    div = np.array(divide_num, dtype=np.float32).reshape(N, 1)
    div[div == 0] = 1.0
    nzb = (index_mask.reshape(NBLK, P, NBLK, P) != 0).any(axis=(1, 3))

    uniq, wlist, blocks = {}, [], {}
    zero_uid = None
    for m in range(NBLK):
        lst = []
        for mj in range(NBLK):
            if not nzb[m, mj]:
                continue
            blk = index_mask[m * P : (m + 1) * P, mj * P : (mj + 1) * P]
            wT = np.ascontiguousarray(
                (blk / div[m * P : (m + 1) * P]).T.astype(np.float16)
            )
            key = wT.tobytes()
            uid = uniq.get(key)
            if uid is None:
                uid = uniq[key] = len(wlist)
                wlist.append(wT)
            lst.append((mj, uid))
        if not lst:  # all-zero mask row: emit one zero matmul so out[m] = 0
            if zero_uid is None:
                zero_uid = len(wlist)
                wlist.append(np.zeros((P, P), np.float16))
            lst.append((m, zero_uid))
        blocks[m] = lst
    return blocks, np.stack(wlist)


def kernel(feats, index_mask, divide_num, _trace=False):
    global LAST
    feats = np.asarray(feats)
    index_mask = np.asarray(index_mask, dtype=np.float32)
    divide_num = np.asarray(divide_num, dtype=np.float32)

    blocks, wstack = _prep_weights(index_mask, divide_num)
    nc = _build(blocks, wstack.shape[0])

    feats16 = np.ascontiguousarray(feats.astype(np.float16))
    in_maps = [
        {"feats": feats16[i * BPC : (i + 1) * BPC], "wgts": wstack}
        for i in range(NCORES)
    ]
    LAST = bass_utils.run_bass_kernel_spmd(
        nc, in_maps, list(range(NCORES)), trace=_trace
    )
    out16 = np.concatenate([LAST.results[i]["out"] for i in range(NCORES)], axis=0)
    return out16.astype(np.float32)
